# revision 1
# baseline (speedup 1.0000x reference)
"""ConvAConnect TRN2 kernel: per-sample noisy-weight 3x3 conv, data-parallel over 8 cores.

Z[b] = conv2d_valid(X[b], W * Werr[loc_id[b]]) + bias * Berr[loc_id[b]]

Shapes: X[32,64,64,64] f32, W[3,3,64,128], bias[128], Werr[1000,3,3,64,128],
Berr[1000,128], loc_id[32] i32 -> Z[32,62,62,128] f32.

Strategy: shard batch (4 samples/core). Host prep = layout only (X transpose
to cin-major, gather of the 32 needed Werr/Berr pool rows, weight reshapes).
All FLOPs (memW = W*Werr, conv, bias) run on device.

Device kernel per sample:
  - Two stacked SBUF tiles of X^T (cin x H*W grid): XTs1 = [X^T; X^T shifted
    1 pixel], XTs64 = [X^T; X^T shifted 64 pixels]. The 9 conv taps collapse
    to 5 matmuls per 512-pixel output chunk: 4 with K=128 (tap pairs) and one
    K=64 single, accumulated in PSUM. fp32r dtype: full fp32 bits in memory,
    relaxed PE multiply (~2e-4 rel err), 2 cyc/row at N=512.
  - Output grid is 62 rows x 64 cols (2 junk columns keep width-64 alignment
    so every tap is a constant offset); junk columns are dropped at DMA time.
  - ScalarE copies PSUM->SBUF fused with the per-sample bias add; TensorE
    transposes [cout, spatial] -> [spatial, cout] in 128x128 blocks; the
    per-sample result is shipped with two 3D-AP DMAs (even/odd output rows),
    one on each HWDGE ring (sync + scalar).
"""

import sys
import numpy as np

for _p in ("/opt/trn_rl_repo", "/root/.axon_site"):
    if _p not in sys.path:
        sys.path.insert(0, _p)

N_CORES = 8
B = 32
PER_CORE = B // N_CORES
H = Wd = 64
CIN = 64
COUT = 128
HO = WO = 62
GRID = HO * 64          # 62 rows x 64 cols (2 junk cols/row)
XTL = 4104              # X^T free length: 4096 valid + pad (max read 4098)
NCHUNK = 512            # output-grid pixels per PSUM chunk (8 grid rows)
NCHUNKS = 8             # 7 full chunks + 1 of 384

_compiled = {}


def _build():
    import concourse.bass as bass
    import concourse.mybir as mybir
    import concourse.tile as tile
    from concourse import bacc
    from concourse.masks import make_identity

    f32 = mybir.dt.float32
    f32r = mybir.dt.float32r

    nc = bacc.Bacc("TRN2", target_bir_lowering=False, debug=False)

    xt_in = nc.dram_tensor("xt", [PER_CORE, CIN, XTL], f32r, kind="ExternalInput")
    wp_in = nc.dram_tensor("wp", [128, 3 * COUT], f32r, kind="ExternalInput")
    wq_in = nc.dram_tensor("wq", [128, COUT], f32r, kind="ExternalInput")
    ws_in = nc.dram_tensor("ws", [64, COUT], f32r, kind="ExternalInput")
    gp_in = nc.dram_tensor("gp", [PER_CORE, 128, 3 * COUT], f32r, kind="ExternalInput")
    gq_in = nc.dram_tensor("gq", [PER_CORE, 128, COUT], f32r, kind="ExternalInput")
    gs_in = nc.dram_tensor("gs", [PER_CORE, 64, COUT], f32r, kind="ExternalInput")
    bias_in = nc.dram_tensor("bias", [COUT, 1], f32, kind="ExternalInput")
    berr_in = nc.dram_tensor("berr", [COUT, PER_CORE], f32, kind="ExternalInput")
    z_out = nc.dram_tensor("z", [PER_CORE, 128, 31 * 128], f32, kind="ExternalOutput")

    with tile.TileContext(nc) as tc:
        with (
            tc.tile_pool(name="const", bufs=1) as const,
            tc.tile_pool(name="xpool", bufs=2) as xpool,
            tc.tile_pool(name="wpool", bufs=2) as wpool,
            tc.tile_pool(name="spool", bufs=4) as spool,
            tc.tile_pool(name="zpool", bufs=2) as zpool,
            tc.tile_pool(name="psmm", bufs=3, space="PSUM") as psmm,
            tc.tile_pool(name="pst", bufs=4, space="PSUM") as pst,
        ):
            identity = const.tile([128, 128], f32, tag="identity")
            make_identity(nc, identity[:])

            wp_t = const.tile([128, 3 * COUT], f32r, tag="wp")
            wq_t = const.tile([128, COUT], f32r, tag="wq")
            ws_t = const.tile([64, COUT], f32r, tag="ws")
            bias_t = const.tile([COUT, 1], f32, tag="bias")
            berr_t = const.tile([COUT, PER_CORE], f32, tag="berr")
            mb_all = const.tile([COUT, PER_CORE], f32, tag="mb")
            nc.sync.dma_start(wp_t[:], wp_in[:])
            nc.sync.dma_start(wq_t[:], wq_in[:])
            nc.sync.dma_start(ws_t[:], ws_in[:])
            nc.sync.dma_start(bias_t[:], bias_in[:])
            nc.sync.dma_start(berr_t[:], berr_in[:])
            nc.vector.tensor_scalar_mul(mb_all[:], berr_t[:], bias_t[:])

            def load_sample(b):
                """DMA the X stacks + noisy-weight factors and form memW."""
                xts1 = xpool.tile([128, XTL], f32r, tag="xts1")
                nc.sync.dma_start(xts1[0:64, :], xt_in[b])
                nc.scalar.dma_start(xts1[64:128, 0 : XTL - 1], xt_in[b][:, 1:XTL])
                xts64 = xpool.tile([128, XTL], f32r, tag="xts64")
                nc.scalar.dma_start(xts64[0:64, :], xt_in[b])
                nc.sync.dma_start(xts64[64:128, 0 : XTL - 64], xt_in[b][:, 64:XTL])

                gpt = wpool.tile([128, 3 * COUT], f32r, tag="gpt")
                gqt = wpool.tile([128, COUT], f32r, tag="gqt")
                gst = wpool.tile([64, COUT], f32r, tag="gst")
                nc.sync.dma_start(gpt[:], gp_in[b])
                nc.scalar.dma_start(gqt[:], gq_in[b])
                nc.scalar.dma_start(gst[:], gs_in[b])
                mwp = wpool.tile([128, 3 * COUT], f32r, tag="mwp")
                mwq = wpool.tile([128, COUT], f32r, tag="mwq")
                mws = wpool.tile([64, COUT], f32r, tag="mws")
                nc.vector.tensor_mul(mwp[:], wp_t[:], gpt[:])
                nc.vector.tensor_mul(mwq[:], wq_t[:], gqt[:])
                nc.vector.tensor_mul(mws[:], ws_t[:], gst[:])
                return xts1, xts64, mwp, mwq, mws

            cur = load_sample(0)
            for b in range(PER_CORE):
                xts1, xts64, mwp, mwq, mws = cur
                # prefetch next sample's inputs ahead of this sample's
                # output scatter so they don't queue behind it on the rings
                if b + 1 < PER_CORE:
                    cur = load_sample(b + 1)

                zbuf = zpool.tile([128, 31 * 128], f32, tag="zbuf")

                for c in range(NCHUNKS):
                    base = c * NCHUNK
                    ncols = min(NCHUNK, GRID - base)
                    pc = psmm.tile([128, NCHUNK], f32, tag="pc")
                    # taps (fh,0)+(fh,1): K=128 pairs from the shift-1 stack
                    for fh in range(3):
                        nc.tensor.matmul(
                            pc[:, :ncols],
                            mwp[:, fh * COUT : (fh + 1) * COUT],
                            xts1[:, base + fh * 64 : base + fh * 64 + ncols],
                            start=(fh == 0),
                            stop=False,
                        )
                    # taps (0,2)+(1,2): K=128 pair from the shift-64 stack
                    nc.tensor.matmul(
                        pc[:, :ncols],
                        mwq[:],
                        xts64[:, base + 2 : base + 2 + ncols],
                        start=False,
                        stop=False,
                    )
                    # tap (2,2): K=64 single (top half of xts1 is unshifted X^T)
                    nc.tensor.matmul(
                        pc[:, :ncols],
                        mws[:],
                        xts1[0:64, base + 130 : base + 130 + ncols],
                        start=False,
                        stop=True,
                    )
                    out_s = spool.tile([128, NCHUNK], f32, tag="outs")
                    nc.scalar.activation(
                        out_s[:, :ncols],
                        pc[:, :ncols],
                        mybir.ActivationFunctionType.Identity,
                        bias=mb_all[:, b : b + 1],
                    )
                    for k in range(ncols // 128):
                        pt = pst.tile([128, 128], f32, tag="pt")
                        nc.tensor.transpose(
                            pt[:], out_s[:, k * 128 : (k + 1) * 128], identity[:]
                        )
                        j = c * 4 + k  # grid row-pair index, 0..30
                        nc.vector.tensor_copy(
                            zbuf[:, j * 128 : (j + 1) * 128], pt[:]
                        )

                # ship the sample as one contiguous DMA in tiled layout
                # (partition p = 64*(ho%2)+wo, free = 128*(ho//2)+cout);
                # the host unshard does the final reshape/junk-drop
                eng = nc.sync if b % 2 == 0 else nc.scalar
                eng.dma_start(z_out[b], zbuf[:])

    nc.compile()
    return nc


def _get_nc():
    if "nc" not in _compiled:
        _compiled["nc"] = _build()
    return _compiled["nc"]


def _prep_inputs(X, W, bias, Werr, Berr, loc_id):
    """Host-side shard/layout prep. Returns per-core in_maps."""
    X = np.asarray(X, dtype=np.float32)
    W = np.asarray(W, dtype=np.float32)
    bias = np.asarray(bias, dtype=np.float32)
    Werr = np.asarray(Werr, dtype=np.float32)
    Berr = np.asarray(Berr, dtype=np.float32)
    loc_id = np.asarray(loc_id)

    # X^T: [B, CIN, H*W] padded to XTL
    xt = np.zeros((B, CIN, XTL), dtype=np.float32)
    xt[:, :, : H * Wd] = X.transpose(0, 3, 1, 2).reshape(B, CIN, H * Wd)

    # wp[fw*64+cin, fh*128+cout] = W[fh, fw, cin, cout] for fw in {0,1}
    wp = np.ascontiguousarray(W[:, :2].transpose(1, 2, 0, 3).reshape(128, 3 * COUT))
    # wq[fh*64+cin, cout] = W[fh, 2, cin, cout] for fh in {0,1}
    wq = np.ascontiguousarray(W[:2, 2].reshape(128, COUT))
    # ws[cin, cout] = W[2, 2, cin, cout]
    ws = np.ascontiguousarray(W[2, 2])

    g = Werr[loc_id]  # [B, 3, 3, 64, 128]
    gp = np.ascontiguousarray(
        g[:, :, :2].transpose(0, 2, 3, 1, 4).reshape(B, 128, 3 * COUT)
    )
    gq = np.ascontiguousarray(g[:, :2, 2].reshape(B, 128, COUT))
    gs = np.ascontiguousarray(g[:, 2, 2])

    be = Berr[loc_id]  # [B, 128]
    bias_col = np.ascontiguousarray(bias.reshape(COUT, 1))

    in_maps = []
    for i in range(N_CORES):
        s = slice(i * PER_CORE, (i + 1) * PER_CORE)
        in_maps.append(
            {
                "xt": np.ascontiguousarray(xt[s]),
                "wp": wp,
                "wq": wq,
                "ws": ws,
                "gp": np.ascontiguousarray(gp[s]),
                "gq": np.ascontiguousarray(gq[s]),
                "gs": np.ascontiguousarray(gs[s]),
                "bias": bias_col,
                "berr": np.ascontiguousarray(be[s].T),
            }
        )
    return in_maps


def _run(in_maps, trace=False, **kw):
    from concourse.bass_utils import run_bass_kernel_spmd

    nc = _get_nc()
    return run_bass_kernel_spmd(nc, in_maps, list(range(N_CORES)), trace=trace, **kw)


def _unshard(results):
    zb = np.concatenate([results[i]["z"] for i in range(N_CORES)], axis=0)
    # zb[b, 64*(ho%2)+wo, 128*(ho//2)+cout] -> Z[b, ho, wo, cout]
    v = zb.reshape(B, 2, 64, 31, COUT).transpose(0, 3, 1, 2, 4).reshape(B, HO, 64, COUT)
    return np.ascontiguousarray(v[:, :, :WO, :])


def kernel(X, W, bias, Werr, Berr, loc_id):
    in_maps = _prep_inputs(X, W, bias, Werr, Berr, loc_id)
    res = _run(in_maps)
    return _unshard(res.results)



# revision 5
# speedup vs baseline: 1.7533x; 1.7533x over previous
"""ConvAConnect TRN2 kernel: per-sample noisy-weight 3x3 conv, data-parallel over 8 cores.

Z[b] = conv2d_valid(X[b], W * Werr[loc_id[b]]) + bias * Berr[loc_id[b]]

Shapes: X[32,64,64,64] f32, W[3,3,64,128], bias[128], Werr[1000,3,3,64,128],
Berr[1000,128], loc_id[32] i32 -> Z[32,62,62,128] f32.

Strategy: shard batch (4 samples/core). Host prep = layout only (X transpose
to cin-major fp16, gather of the 32 needed Werr/Berr pool rows, weight
reshapes). All FLOPs (memW = W*Werr, conv, bias) run on device.

Device kernel per sample (fp16 operands, f32 PSUM accumulate):
  - Two stacked SBUF tiles of X^T (cin x H*W grid): XTs1 = [X^T; X^T shifted
    1 pixel], XTs64 = [X^T; X^T shifted 64 pixels]. The 9 conv taps collapse
    to 5 matmuls per 512-pixel output chunk: 4 with K=128 (tap pairs) and one
    K=64 single, accumulated in PSUM. fp16 runs the PE at 1 cyc/row (2x the
    fp32r rate) and halves all DMA traffic; quantization error ~4e-4 rel.
  - Output grid is 62 rows x 64 cols (2 junk columns keep width-64 alignment
    so every tap is a constant offset); junk columns are dropped on host.
  - ScalarE drains PSUM->SBUF fused with the per-sample bias add, emitting
    fp16 into a [cout, grid] zbuf (no on-device transpose; host transposes).
  - Weight factors ship as one concatenated [128, 640] tile per sample.
  - DMA issue is spread across the sync/gpsimd/vector rings so ScalarE only
    runs ACTIVATE and TensorE only runs matmuls.
"""

import sys
import numpy as np

for _p in ("/opt/trn_rl_repo", "/root/.axon_site"):
    if _p not in sys.path:
        sys.path.insert(0, _p)

N_CORES = 8
B = 32
PER_CORE = B // N_CORES
H = Wd = 64
CIN = 64
COUT = 128
HO = WO = 62
GRID = HO * 64          # 62 rows x 64 cols (2 junk cols/row)
XTL = 4104              # X^T free length: 4096 valid + pad (max read 4098)
NCHUNK = 512            # output-grid pixels per PSUM chunk (8 grid rows)
NCHUNKS = 8             # 7 full chunks + 1 of 384
WCAT = 5 * COUT         # wp(3*128) | wq(128) | ws(128)

_compiled = {}


def _build():
    import concourse.bass as bass
    import concourse.mybir as mybir
    import concourse.tile as tile
    from concourse import bacc

    f32 = mybir.dt.float32
    f16 = mybir.dt.float16

    nc = bacc.Bacc("TRN2", target_bir_lowering=False, debug=False)

    xt_in = nc.dram_tensor("xt", [PER_CORE, CIN, XTL], f16, kind="ExternalInput")
    wcat_in = nc.dram_tensor("wcat", [128, WCAT], f16, kind="ExternalInput")
    gcat_in = nc.dram_tensor("gcat", [PER_CORE, 128, WCAT], f16, kind="ExternalInput")
    bias_in = nc.dram_tensor("bias", [COUT, 1], f32, kind="ExternalInput")
    berr_in = nc.dram_tensor("berr", [COUT, PER_CORE], f32, kind="ExternalInput")
    z_out = nc.dram_tensor("z", [PER_CORE, 128, GRID], f16, kind="ExternalOutput")

    with tile.TileContext(nc) as tc:
        with (
            tc.tile_pool(name="const", bufs=1) as const,
            tc.tile_pool(name="xpool", bufs=2) as xpool,
            tc.tile_pool(name="wpool", bufs=2) as wpool,
            tc.tile_pool(name="zpool", bufs=2) as zpool,
            tc.tile_pool(name="psmm", bufs=4, space="PSUM") as psmm,
        ):
            wcat_t = const.tile([128, WCAT], f16, tag="wcat")
            bias_t = const.tile([COUT, 1], f32, tag="bias")
            berr_t = const.tile([COUT, PER_CORE], f32, tag="berr")
            mb_all = const.tile([COUT, PER_CORE], f32, tag="mb")
            nc.sync.dma_start(wcat_t[:], wcat_in[:])
            nc.sync.dma_start(bias_t[:], bias_in[:])
            nc.sync.dma_start(berr_t[:], berr_in[:])
            nc.vector.tensor_scalar_mul(mb_all[:], berr_t[:], bias_t[:])

            def load_sample(b):
                """DMA the X stacks + noisy-weight factors and form memW."""
                xts1 = xpool.tile([128, XTL], f16, tag="xts1")
                nc.sync.dma_start(xts1[0:64, :], xt_in[b])
                nc.gpsimd.dma_start(xts1[64:128, 0 : XTL - 1], xt_in[b][:, 1:XTL])
                xts64 = xpool.tile([128, XTL], f16, tag="xts64")
                nc.scalar.dma_start(xts64[0:64, :], xt_in[b])
                nc.sync.dma_start(xts64[64:128, 0 : XTL - 64], xt_in[b][:, 64:XTL])

                gcat = wpool.tile([128, WCAT], f16, tag="gcat")
                nc.gpsimd.dma_start(gcat[:], gcat_in[b])
                mw = wpool.tile([128, WCAT], f16, tag="mw")
                nc.vector.tensor_mul(mw[:], wcat_t[:], gcat[:])
                return xts1, xts64, mw

            cur = load_sample(0)
            for b in range(PER_CORE):
                xts1, xts64, mw = cur
                # prefetch next sample's inputs ahead of this sample's
                # output DMA so they don't queue behind it on the rings
                if b + 1 < PER_CORE:
                    cur = load_sample(b + 1)

                zbuf = zpool.tile([128, GRID], f16, tag="zbuf")

                for c in range(NCHUNKS):
                    base = c * NCHUNK
                    ncols = min(NCHUNK, GRID - base)
                    pc = psmm.tile([128, NCHUNK], f32, tag="pc")
                    # taps (fh,0)+(fh,1): K=128 pairs from the shift-1 stack
                    for fh in range(3):
                        nc.tensor.matmul(
                            pc[:, :ncols],
                            mw[:, fh * COUT : (fh + 1) * COUT],
                            xts1[:, base + fh * 64 : base + fh * 64 + ncols],
                            start=(fh == 0),
                            stop=False,
                        )
                    # taps (0,2)+(1,2): K=128 pair from the shift-64 stack
                    nc.tensor.matmul(
                        pc[:, :ncols],
                        mw[:, 3 * COUT : 4 * COUT],
                        xts64[:, base + 2 : base + 2 + ncols],
                        start=False,
                        stop=False,
                    )
                    # tap (2,2): K=64 single (top half of xts1 is unshifted X^T)
                    nc.tensor.matmul(
                        pc[:, :ncols],
                        mw[0:64, 4 * COUT : 5 * COUT],
                        xts1[0:64, base + 130 : base + 130 + ncols],
                        start=False,
                        stop=True,
                    )
                    # drain PSUM -> zbuf fused with the per-sample bias add
                    nc.scalar.activation(
                        zbuf[:, base : base + ncols],
                        pc[:, :ncols],
                        mybir.ActivationFunctionType.Identity,
                        bias=mb_all[:, b : b + 1],
                    )

                # ship the sample as one contiguous [cout, grid] DMA;
                # the host unshard does the final transpose/junk-drop
                eng = nc.sync if b % 2 == 0 else nc.scalar
                eng.dma_start(z_out[b], zbuf[:])

    nc.compile()
    return nc


def _get_nc():
    if "nc" not in _compiled:
        _compiled["nc"] = _build()
    return _compiled["nc"]


def _prep_inputs(X, W, bias, Werr, Berr, loc_id):
    """Host-side shard/layout prep. Returns per-core in_maps."""
    X = np.asarray(X, dtype=np.float32)
    W = np.asarray(W, dtype=np.float32)
    bias = np.asarray(bias, dtype=np.float32)
    Werr = np.asarray(Werr, dtype=np.float32)
    Berr = np.asarray(Berr, dtype=np.float32)
    loc_id = np.asarray(loc_id)

    # X^T: [B, CIN, H*W] padded to XTL, fp16
    xt = np.zeros((B, CIN, XTL), dtype=np.float16)
    xt[:, :, : H * Wd] = X.transpose(0, 3, 1, 2).reshape(B, CIN, H * Wd)

    # wcat[128, 640] = wp(3*128) | wq(128) | ws(128)
    # wp[fw*64+cin, fh*128+cout] = W[fh, fw, cin, cout] for fw in {0,1}
    # wq[fh*64+cin, cout] = W[fh, 2, cin, cout] for fh in {0,1}
    # ws[cin, cout] = W[2, 2, cin, cout] (rows 64:128 zero)
    def cat_blocks(w):
        lead = w.shape[:-4]
        out = np.zeros(lead + (128, WCAT), dtype=np.float16)
        # [..., fh, fw2, cin, cout] -> [..., fw2, cin, fh, cout] -> [128, 384]
        out[..., :, 0 : 3 * COUT] = np.moveaxis(w[..., :, :2, :, :], -4, -2).reshape(
            lead + (128, 3 * COUT)
        )
        # [..., fh2, cin, cout] -> [128, 128] with p = fh*64+cin
        out[..., :, 3 * COUT : 4 * COUT] = w[..., :2, 2, :, :].reshape(
            lead + (128, COUT)
        )
        out[..., 0:64, 4 * COUT : 5 * COUT] = w[..., 2, 2, :, :]
        return out

    wcat = cat_blocks(W)                      # [128, 640]
    gcat = cat_blocks(Werr[loc_id])           # [B, 128, 640]

    be = Berr[loc_id]  # [B, 128]
    bias_col = np.ascontiguousarray(bias.reshape(COUT, 1))

    in_maps = []
    for i in range(N_CORES):
        s = slice(i * PER_CORE, (i + 1) * PER_CORE)
        in_maps.append(
            {
                "xt": np.ascontiguousarray(xt[s]),
                "wcat": wcat,
                "gcat": np.ascontiguousarray(gcat[s]),
                "bias": bias_col,
                "berr": np.ascontiguousarray(be[s].T),
            }
        )
    return in_maps


def _run(in_maps, trace=False, **kw):
    from concourse.bass_utils import run_bass_kernel_spmd

    nc = _get_nc()
    return run_bass_kernel_spmd(nc, in_maps, list(range(N_CORES)), trace=trace, **kw)


def _unshard(results):
    zb = np.concatenate([results[i]["z"] for i in range(N_CORES)], axis=0)
    # zb[b, cout, ho*64+wo] -> Z[b, ho, wo, cout]
    v = zb.astype(np.float32).reshape(B, COUT, HO, 64).transpose(0, 2, 3, 1)
    return np.ascontiguousarray(v[:, :, :WO, :])


def kernel(X, W, bias, Werr, Berr, loc_id):
    in_maps = _prep_inputs(X, W, bias, Werr, Berr, loc_id)
    res = _run(in_maps)
    return _unshard(res.results)


# revision 6
# speedup vs baseline: 1.8234x; 1.0400x over previous
"""ConvAConnect TRN2 kernel: per-sample noisy-weight 3x3 conv, data-parallel over 8 cores.

Z[b] = conv2d_valid(X[b], W * Werr[loc_id[b]]) + bias * Berr[loc_id[b]]

Shapes: X[32,64,64,64] f32, W[3,3,64,128], bias[128], Werr[1000,3,3,64,128],
Berr[1000,128], loc_id[32] i32 -> Z[32,62,62,128] f32.

Strategy: shard batch (4 samples/core). Host prep = layout only (X transpose
to cin-major fp16, gather of the 32 needed Werr/Berr pool rows, weight
reshapes). All FLOPs (memW = W*Werr, conv, bias) run on device.

Device kernel per sample (fp16 operands, f32 PSUM accumulate):
  - Two stacked SBUF tiles of X^T (cin x H*W grid): XTs1 = [X^T; X^T shifted
    1 pixel], XTs64 = [X^T; X^T shifted 64 pixels]. The 9 conv taps collapse
    to 5 matmuls per 512-pixel output chunk: 4 with K=128 (tap pairs) and one
    K=64 single, accumulated in PSUM. fp16 runs the PE at 1 cyc/row (2x the
    fp32r rate) and halves all DMA traffic; quantization error ~4e-4 rel.
  - Output grid is 62 rows x 64 cols (2 junk columns keep width-64 alignment
    so every tap is a constant offset); junk columns are dropped on host.
  - ScalarE drains PSUM->SBUF fused with the per-sample bias add, emitting
    fp16 into a [cout, grid] zbuf (no on-device transpose; host transposes).
  - Weight factors ship as one concatenated [128, 640] tile per sample.
  - DMA issue is spread across the sync/gpsimd/vector rings so ScalarE only
    runs ACTIVATE and TensorE only runs matmuls.
"""

import sys
import numpy as np

for _p in ("/opt/trn_rl_repo", "/root/.axon_site"):
    if _p not in sys.path:
        sys.path.insert(0, _p)

N_CORES = 8
B = 32
PER_CORE = B // N_CORES
H = Wd = 64
CIN = 64
COUT = 128
HO = WO = 62
GRID = HO * 64          # 62 rows x 64 cols (2 junk cols/row)
XTL = 4104              # X^T free length: 4096 valid + pad (max read 4098)
NCHUNK = 512            # output-grid pixels per PSUM chunk (8 grid rows)
NCHUNKS = 8             # 7 full chunks + 1 of 384
WCAT = 5 * COUT         # wp(3*128) | wq(128) | ws(128)

_compiled = {}


def _build():
    import concourse.bass as bass
    import concourse.mybir as mybir
    import concourse.tile as tile
    from concourse import bacc

    f32 = mybir.dt.float32
    f16 = mybir.dt.float16

    nc = bacc.Bacc("TRN2", target_bir_lowering=False, debug=False)

    xt_in = nc.dram_tensor("xt", [PER_CORE, CIN, XTL], f16, kind="ExternalInput")
    wcat_in = nc.dram_tensor("wcat", [128, WCAT], f16, kind="ExternalInput")
    gcat_in = nc.dram_tensor("gcat", [PER_CORE, 128, WCAT], f16, kind="ExternalInput")
    bias_in = nc.dram_tensor("bias", [COUT, 1], f32, kind="ExternalInput")
    berr_in = nc.dram_tensor("berr", [COUT, PER_CORE], f32, kind="ExternalInput")
    z_out = nc.dram_tensor("z", [PER_CORE, 128, GRID], f16, kind="ExternalOutput")

    with tile.TileContext(nc) as tc:
        with (
            tc.tile_pool(name="const", bufs=1) as const,
            tc.tile_pool(name="xpool", bufs=3) as xpool,
            tc.tile_pool(name="wpool", bufs=3) as wpool,
            tc.tile_pool(name="zpool", bufs=3) as zpool,
            tc.tile_pool(name="psmm", bufs=4, space="PSUM") as psmm,
        ):
            wcat_t = const.tile([128, WCAT], f16, tag="wcat")
            bias_t = const.tile([COUT, 1], f32, tag="bias")
            berr_t = const.tile([COUT, PER_CORE], f32, tag="berr")
            mb_all = const.tile([COUT, PER_CORE], f32, tag="mb")
            nc.scalar.dma_start(wcat_t[:], wcat_in[:])
            nc.sync.dma_start(bias_t[:], bias_in[:])
            nc.sync.dma_start(berr_t[:], berr_in[:])
            nc.vector.tensor_scalar_mul(mb_all[:], berr_t[:], bias_t[:])

            XH = 2052  # column split point for dual-ring X loads

            def load_sample(b):
                """DMA the X stacks + noisy-weight factors and form memW.

                Each X half-load is split into two column segments on
                different rings so a single sample's load latency is
                spread across all three HWDGE queues."""
                gcat = wpool.tile([128, WCAT], f16, tag="gcat")
                nc.gpsimd.dma_start(gcat[:], gcat_in[b])
                mw = wpool.tile([128, WCAT], f16, tag="mw")
                nc.vector.tensor_mul(mw[:], wcat_t[:], gcat[:])

                xts1 = xpool.tile([128, XTL], f16, tag="xts1")
                nc.sync.dma_start(xts1[0:64, 0:XH], xt_in[b][:, 0:XH])
                nc.scalar.dma_start(xts1[0:64, XH:XTL], xt_in[b][:, XH:XTL])
                nc.gpsimd.dma_start(xts1[64:128, 0:XH], xt_in[b][:, 1 : XH + 1])
                nc.sync.dma_start(
                    xts1[64:128, XH : XTL - 1], xt_in[b][:, XH + 1 : XTL]
                )
                xts64 = xpool.tile([128, XTL], f16, tag="xts64")
                nc.scalar.dma_start(xts64[0:64, 0:XH], xt_in[b][:, 0:XH])
                nc.gpsimd.dma_start(xts64[0:64, XH:XTL], xt_in[b][:, XH:XTL])
                nc.sync.dma_start(xts64[64:128, 0:XH], xt_in[b][:, 64 : XH + 64])
                nc.scalar.dma_start(
                    xts64[64:128, XH : XTL - 64], xt_in[b][:, XH + 64 : XTL]
                )
                return xts1, xts64, mw

            ZH = 4 * NCHUNK  # output ships in two halves, after chunks 3 and 7

            samples = [load_sample(0), load_sample(1)]
            for b in range(PER_CORE):
                xts1, xts64, mw = samples[b]
                # prefetch two samples ahead of this sample's output DMA
                # so loads don't queue behind it on the rings
                if b + 2 < PER_CORE:
                    samples.append(load_sample(b + 2))

                zbuf = zpool.tile([128, GRID], f16, tag="zbuf")

                for c in range(NCHUNKS):
                    base = c * NCHUNK
                    ncols = min(NCHUNK, GRID - base)
                    pc = psmm.tile([128, NCHUNK], f32, tag="pc")
                    # taps (fh,0)+(fh,1): K=128 pairs from the shift-1 stack
                    for fh in range(3):
                        nc.tensor.matmul(
                            pc[:, :ncols],
                            mw[:, fh * COUT : (fh + 1) * COUT],
                            xts1[:, base + fh * 64 : base + fh * 64 + ncols],
                            start=(fh == 0),
                            stop=False,
                        )
                    # taps (0,2)+(1,2): K=128 pair from the shift-64 stack
                    nc.tensor.matmul(
                        pc[:, :ncols],
                        mw[:, 3 * COUT : 4 * COUT],
                        xts64[:, base + 2 : base + 2 + ncols],
                        start=False,
                        stop=False,
                    )
                    # tap (2,2): K=64 single (top half of xts1 is unshifted X^T)
                    nc.tensor.matmul(
                        pc[:, :ncols],
                        mw[0:64, 4 * COUT : 5 * COUT],
                        xts1[0:64, base + 130 : base + 130 + ncols],
                        start=False,
                        stop=True,
                    )
                    # drain PSUM -> zbuf fused with the per-sample bias add
                    nc.scalar.activation(
                        zbuf[:, base : base + ncols],
                        pc[:, :ncols],
                        mybir.ActivationFunctionType.Identity,
                        bias=mb_all[:, b : b + 1],
                    )
                    if c == 3:
                        # first half is drained; ship it while the second
                        # half computes (host does the final transpose)
                        eng = nc.sync if b % 2 == 0 else nc.scalar
                        eng.dma_start(z_out[b][:, 0:ZH], zbuf[:, 0:ZH])

                eng = nc.scalar if b % 2 == 0 else nc.sync
                eng.dma_start(z_out[b][:, ZH:GRID], zbuf[:, ZH:GRID])

    nc.compile()
    return nc


def _get_nc():
    if "nc" not in _compiled:
        _compiled["nc"] = _build()
    return _compiled["nc"]


def _prep_inputs(X, W, bias, Werr, Berr, loc_id):
    """Host-side shard/layout prep. Returns per-core in_maps."""
    X = np.asarray(X, dtype=np.float32)
    W = np.asarray(W, dtype=np.float32)
    bias = np.asarray(bias, dtype=np.float32)
    Werr = np.asarray(Werr, dtype=np.float32)
    Berr = np.asarray(Berr, dtype=np.float32)
    loc_id = np.asarray(loc_id)

    # X^T: [B, CIN, H*W] padded to XTL, fp16
    xt = np.zeros((B, CIN, XTL), dtype=np.float16)
    xt[:, :, : H * Wd] = X.transpose(0, 3, 1, 2).reshape(B, CIN, H * Wd)

    # wcat[128, 640] = wp(3*128) | wq(128) | ws(128)
    # wp[fw*64+cin, fh*128+cout] = W[fh, fw, cin, cout] for fw in {0,1}
    # wq[fh*64+cin, cout] = W[fh, 2, cin, cout] for fh in {0,1}
    # ws[cin, cout] = W[2, 2, cin, cout] (rows 64:128 zero)
    def cat_blocks(w):
        lead = w.shape[:-4]
        out = np.zeros(lead + (128, WCAT), dtype=np.float16)
        # [..., fh, fw2, cin, cout] -> [..., fw2, cin, fh, cout] -> [128, 384]
        out[..., :, 0 : 3 * COUT] = np.moveaxis(w[..., :, :2, :, :], -4, -2).reshape(
            lead + (128, 3 * COUT)
        )
        # [..., fh2, cin, cout] -> [128, 128] with p = fh*64+cin
        out[..., :, 3 * COUT : 4 * COUT] = w[..., :2, 2, :, :].reshape(
            lead + (128, COUT)
        )
        out[..., 0:64, 4 * COUT : 5 * COUT] = w[..., 2, 2, :, :]
        return out

    wcat = cat_blocks(W)                      # [128, 640]
    gcat = cat_blocks(Werr[loc_id])           # [B, 128, 640]

    be = Berr[loc_id]  # [B, 128]
    bias_col = np.ascontiguousarray(bias.reshape(COUT, 1))

    in_maps = []
    for i in range(N_CORES):
        s = slice(i * PER_CORE, (i + 1) * PER_CORE)
        in_maps.append(
            {
                "xt": np.ascontiguousarray(xt[s]),
                "wcat": wcat,
                "gcat": np.ascontiguousarray(gcat[s]),
                "bias": bias_col,
                "berr": np.ascontiguousarray(be[s].T),
            }
        )
    return in_maps


def _run(in_maps, trace=False, **kw):
    from concourse.bass_utils import run_bass_kernel_spmd

    nc = _get_nc()
    return run_bass_kernel_spmd(nc, in_maps, list(range(N_CORES)), trace=trace, **kw)


def _unshard(results):
    zb = np.concatenate([results[i]["z"] for i in range(N_CORES)], axis=0)
    # zb[b, cout, ho*64+wo] -> Z[b, ho, wo, cout]
    v = zb.astype(np.float32).reshape(B, COUT, HO, 64).transpose(0, 2, 3, 1)
    return np.ascontiguousarray(v[:, :, :WO, :])


def kernel(X, W, bias, Werr, Berr, loc_id):
    in_maps = _prep_inputs(X, W, bias, Werr, Berr, loc_id)
    res = _run(in_maps)
    return _unshard(res.results)


# revision 7
# speedup vs baseline: 2.0789x; 1.1401x over previous
"""ConvAConnect TRN2 kernel: per-sample noisy-weight 3x3 conv, data-parallel over 8 cores.

Z[b] = conv2d_valid(X[b], W * Werr[loc_id[b]]) + bias * Berr[loc_id[b]]

Shapes: X[32,64,64,64] f32, W[3,3,64,128], bias[128], Werr[1000,3,3,64,128],
Berr[1000,128], loc_id[32] i32 -> Z[32,62,62,128] f32.

Strategy: shard batch (4 samples/core). Per the sharding hint, the per-sample
noisy weights memW = W*Werr[loc_id] and membias = bias*Berr[loc_id] are formed
host-side and sharded with the batch; X ships as fp16 cin-major X^T.

Device kernel per sample (fp16 operands, f32 PSUM accumulate):
  - Two stacked SBUF tiles of X^T (cin x H*W grid): XTs1 = [X^T; X^T shifted
    1 pixel], XTs64 = [X^T; X^T shifted 64 pixels]. The 9 conv taps collapse
    to 5 matmuls per 512-pixel output chunk, all K=128 (the tap-(2,2) block
    zero-pads its lower 64 weight rows so the PE never switches tile size),
    accumulated in PSUM. fp16 runs the PE at 1 cyc/row.
  - Output grid is 62 rows x 64 cols (2 junk columns keep width-64 alignment
    so every tap is a constant offset); junk columns are dropped on host.
  - ScalarE drains PSUM->SBUF fused with the per-sample bias add, emitting
    fp16 into a [cout, grid] zbuf (no on-device transpose; host transposes).
  - X loads are split column-wise across the sync/scalar/gpsimd rings; each
    sample's output ships in two halves overlapped with compute.
"""

import sys
import numpy as np

for _p in ("/opt/trn_rl_repo", "/root/.axon_site"):
    if _p not in sys.path:
        sys.path.insert(0, _p)

N_CORES = 8
B = 32
PER_CORE = B // N_CORES
H = Wd = 64
CIN = 64
COUT = 128
HO = WO = 62
GRID = HO * 64          # 62 rows x 64 cols (2 junk cols/row)
XTL = 4104              # X^T free length: 4096 valid + pad (max read 4098)
NCHUNK = 512            # output-grid pixels per PSUM chunk (8 grid rows)
NCHUNKS = 8             # 7 full chunks + 1 of 384
WCAT = 5 * COUT         # wp(3*128) | wq(128) | ws(128, lower rows zero)

_compiled = {}


def _build():
    import concourse.mybir as mybir
    import concourse.tile as tile
    from concourse import bacc

    f32 = mybir.dt.float32
    f16 = mybir.dt.float16

    nc = bacc.Bacc("TRN2", target_bir_lowering=False, debug=False)

    xt_in = nc.dram_tensor("xt", [PER_CORE, CIN, XTL], f16, kind="ExternalInput")
    mw_in = nc.dram_tensor("mw", [PER_CORE, 128, WCAT], f16, kind="ExternalInput")
    mb_in = nc.dram_tensor("mb", [COUT, PER_CORE], f32, kind="ExternalInput")
    z_out = nc.dram_tensor("z", [PER_CORE, 128, GRID], f16, kind="ExternalOutput")

    with tile.TileContext(nc) as tc:
        with (
            tc.tile_pool(name="const", bufs=1) as const,
            tc.tile_pool(name="xpool", bufs=3) as xpool,
            tc.tile_pool(name="wpool", bufs=3) as wpool,
            tc.tile_pool(name="zpool", bufs=3) as zpool,
            tc.tile_pool(name="psmm", bufs=4, space="PSUM") as psmm,
        ):
            mb_all = const.tile([COUT, PER_CORE], f32, tag="mb")
            nc.sync.dma_start(mb_all[:], mb_in[:])

            XH = 2052  # column split point for dual-ring X loads

            def load_sample(b):
                """DMA the X stacks + per-sample noisy weights.

                Each X half-load is split into two column segments on
                different rings so a single sample's load latency is
                spread across all three HWDGE queues."""
                mw = wpool.tile([128, WCAT], f16, tag="mw")
                nc.gpsimd.dma_start(mw[:], mw_in[b])

                xts1 = xpool.tile([128, XTL], f16, tag="xts1")
                nc.sync.dma_start(xts1[0:64, 0:XH], xt_in[b][:, 0:XH])
                nc.scalar.dma_start(xts1[0:64, XH:XTL], xt_in[b][:, XH:XTL])
                nc.gpsimd.dma_start(xts1[64:128, 0:XH], xt_in[b][:, 1 : XH + 1])
                nc.sync.dma_start(
                    xts1[64:128, XH : XTL - 1], xt_in[b][:, XH + 1 : XTL]
                )
                xts64 = xpool.tile([128, XTL], f16, tag="xts64")
                nc.scalar.dma_start(xts64[0:64, 0:XH], xt_in[b][:, 0:XH])
                nc.gpsimd.dma_start(xts64[0:64, XH:XTL], xt_in[b][:, XH:XTL])
                nc.sync.dma_start(xts64[64:128, 0:XH], xt_in[b][:, 64 : XH + 64])
                nc.scalar.dma_start(
                    xts64[64:128, XH : XTL - 64], xt_in[b][:, XH + 64 : XTL]
                )
                return xts1, xts64, mw

            ZH = 4 * NCHUNK  # output ships in two halves, after chunks 3 and 7

            samples = [load_sample(0), load_sample(1)]
            for b in range(PER_CORE):
                xts1, xts64, mw = samples[b]
                # prefetch two samples ahead of this sample's output DMA
                # so loads don't queue behind it on the rings
                if b + 2 < PER_CORE:
                    samples.append(load_sample(b + 2))

                zbuf = zpool.tile([128, GRID], f16, tag="zbuf")

                for c in range(NCHUNKS):
                    base = c * NCHUNK
                    ncols = min(NCHUNK, GRID - base)
                    pc = psmm.tile([128, NCHUNK], f32, tag="pc")
                    # taps (fh,0)+(fh,1): K=128 pairs from the shift-1 stack
                    for fh in range(3):
                        nc.tensor.matmul(
                            pc[:, :ncols],
                            mw[:, fh * COUT : (fh + 1) * COUT],
                            xts1[:, base + fh * 64 : base + fh * 64 + ncols],
                            start=(fh == 0),
                            stop=False,
                        )
                    # taps (0,2)+(1,2): K=128 pair from the shift-64 stack
                    nc.tensor.matmul(
                        pc[:, :ncols],
                        mw[:, 3 * COUT : 4 * COUT],
                        xts64[:, base + 2 : base + 2 + ncols],
                        start=False,
                        stop=False,
                    )
                    # tap (2,2): K=128 with zero lower weight rows (keeps the
                    # PE tile size constant; the X<<1 junk rows hit zeros)
                    nc.tensor.matmul(
                        pc[:, :ncols],
                        mw[:, 4 * COUT : 5 * COUT],
                        xts1[:, base + 130 : base + 130 + ncols],
                        start=False,
                        stop=True,
                    )
                    # drain PSUM -> zbuf fused with the per-sample bias add
                    nc.scalar.activation(
                        zbuf[:, base : base + ncols],
                        pc[:, :ncols],
                        mybir.ActivationFunctionType.Identity,
                        bias=mb_all[:, b : b + 1],
                    )
                    if c == 3:
                        # first half is drained; ship it while the second
                        # half computes (host does the final transpose)
                        eng = nc.sync if b % 2 == 0 else nc.scalar
                        eng.dma_start(z_out[b][:, 0:ZH], zbuf[:, 0:ZH])

                eng = nc.scalar if b % 2 == 0 else nc.sync
                eng.dma_start(z_out[b][:, ZH:GRID], zbuf[:, ZH:GRID])

    nc.compile()
    return nc


def _get_nc():
    if "nc" not in _compiled:
        _compiled["nc"] = _build()
    return _compiled["nc"]


def _prep_inputs(X, W, bias, Werr, Berr, loc_id):
    """Host-side shard/layout prep. Returns per-core in_maps."""
    X = np.asarray(X, dtype=np.float32)
    W = np.asarray(W, dtype=np.float32)
    bias = np.asarray(bias, dtype=np.float32)
    Werr = np.asarray(Werr, dtype=np.float32)
    Berr = np.asarray(Berr, dtype=np.float32)
    loc_id = np.asarray(loc_id)

    # X^T: [B, CIN, H*W] padded to XTL, fp16
    xt = np.zeros((B, CIN, XTL), dtype=np.float16)
    xt[:, :, : H * Wd] = X.transpose(0, 3, 1, 2).reshape(B, CIN, H * Wd)

    # memW = W * Werr[loc_id], laid out as [128, 640]:
    #   wp[fw*64+cin, fh*128+cout] = memW[fh, fw, cin, cout] for fw in {0,1}
    #   wq[fh*64+cin, cout] = memW[fh, 2, cin, cout] for fh in {0,1}
    #   ws[cin, cout] = memW[2, 2, cin, cout] (rows 64:128 zero)
    def cat_blocks(w):
        lead = w.shape[:-4]
        out = np.zeros(lead + (128, WCAT), dtype=np.float16)
        out[..., :, 0 : 3 * COUT] = np.moveaxis(w[..., :, :2, :, :], -4, -2).reshape(
            lead + (128, 3 * COUT)
        )
        out[..., :, 3 * COUT : 4 * COUT] = w[..., :2, 2, :, :].reshape(
            lead + (128, COUT)
        )
        out[..., 0:64, 4 * COUT : 5 * COUT] = w[..., 2, 2, :, :]
        return out

    mwcat = cat_blocks(W[None] * Werr[loc_id])   # [B, 128, 640] fp16
    mb = (bias[None] * Berr[loc_id]).astype(np.float32)  # [B, 128]

    in_maps = []
    for i in range(N_CORES):
        s = slice(i * PER_CORE, (i + 1) * PER_CORE)
        in_maps.append(
            {
                "xt": np.ascontiguousarray(xt[s]),
                "mw": np.ascontiguousarray(mwcat[s]),
                "mb": np.ascontiguousarray(mb[s].T),
            }
        )
    return in_maps


def _run(in_maps, trace=False, **kw):
    from concourse.bass_utils import run_bass_kernel_spmd

    nc = _get_nc()
    return run_bass_kernel_spmd(nc, in_maps, list(range(N_CORES)), trace=trace, **kw)


def _unshard(results):
    zb = np.concatenate([results[i]["z"] for i in range(N_CORES)], axis=0)
    # zb[b, cout, ho*64+wo] -> Z[b, ho, wo, cout]
    v = zb.astype(np.float32).reshape(B, COUT, HO, 64).transpose(0, 2, 3, 1)
    return np.ascontiguousarray(v[:, :, :WO, :])


def kernel(X, W, bias, Werr, Berr, loc_id):
    in_maps = _prep_inputs(X, W, bias, Werr, Berr, loc_id)
    res = _run(in_maps)
    return _unshard(res.results)
